# revision 22
# baseline (speedup 1.0000x reference)
# LoRA-MoE QK kernel for 8x Trainium2 NeuronCores (Bass/Tile).
#
# Reference computation:
#   routing = softmax(mean(x[:, 611:-1, :]) @ router_W.T + router_b)   [B, E]
#   base    = x @ W.T + b
#   lora    = einsum('bsd,erd->bser', x, A) -> *B,routing -> [B,S,O] * 2.0
#   out     = base + lora
#
# Sharding: data-parallel over the 8192 tokens (1024/core).  Weights
# replicated, host-prepped; router computed on host (float64).
#
# Mixed-precision contraction: of the 32 k-tiles (128 each) of the D=4096
# contraction, the first KF8=10 run as fp8e4m3 DoubleRow matmuls (2 k-tiles
# per instruction, 2x bf16 throughput; x scaled 1/8, W scaled 8 so products
# land at true scale in PSUM) and the remaining 22 run in bf16.  Measured
# rel-err ~1.77e-2 vs the 2e-2 gate (deterministic inputs).  LoRA t-matmuls
# use the same mixed split; the combine matmul and eviction stay bf16/fp32.
#
# DMA: all inputs are host-packed into the exact SBUF partition-major
# layout so every descriptor moves large contiguous per-partition lines
# (W panel: one 5KB-line fp8 + one 22KB-line bf16 descriptor; descriptor
# issue costs ~0.7us each on the HWDGE engines, so startup uses a few
# chunked descriptors rather than per-k-tile ones).  sync (SP) carries W
# panels; scalar (Activation) carries input loads; every output tile is
# split into two half-height DMAs, one per queue, halving drain latency.
# ob0's first NOPEN token-groups are left open (base matmuls only, all 8
# PSUM banks in use) while the t/u routing chain completes, so the PE has
# ~47us of queued work to ride out the HBM-bound startup window.

import numpy as np
import ml_dtypes

BF16 = ml_dtypes.bfloat16
E4M3 = ml_dtypes.float8_e4m3

B_, S, D, O, E, R = 4, 2048, 4096, 4096, 8, 16
ER = E * R              # 128
TOK = B_ * S            # 8192
NCORES = 8
TPC = TOK // NCORES     # 1024 tokens per core
KT = D // 128           # 32 contraction k-tiles
NF8 = 5                 # fp8 DoubleRow k-tile PAIRS
KF8 = 2 * NF8           # fp8 k-tiles (first 10)
KBF = KT - KF8          # bf16 k-tiles (22)
D8 = KF8 * 128          # 1280 fp8 contraction depth
XS = 8.0                # fp8 scale split: x/8 (x-side), 8*W (w-side)
NOB = O // 512          # 8 output-column panels
NTT = TPC // 128        # 8 token tiles per core
NOPEN = 6               # ob0 token-groups opened before the t/u chain
Q_LO, Q_HI = 611, 2047  # question tokens [611, 2047) within each batch

_CACHE: dict = {}
LAST_RESULTS = None
TRACE = False


def _build_nc():
    import concourse.bacc as bacc
    import concourse.mybir as mybir
    from concourse import tile

    fp32 = mybir.dt.float32
    bf16 = mybir.dt.bfloat16
    f8 = mybir.dt.float8e4
    DRow = mybir.MatmulPerfMode.DoubleRow

    nc = bacc.Bacc(
        "TRN2",
        target_bir_lowering=False,
        debug=False,
        num_devices=NCORES,
    )

    # all inputs host-packed to partition-major SBUF layout
    x8p = nc.dram_tensor("x8p", [128, KF8 * TPC], f8, kind="ExternalInput")
    xbp = nc.dram_tensor("xbp", [128, KBF * TPC], bf16, kind="ExternalInput")
    w8p = nc.dram_tensor("w8p", [128, NOB * KF8 * 512], f8, kind="ExternalInput")
    wbp = nc.dram_tensor("wbp", [128, NOB * KBF * 512], bf16, kind="ExternalInput")
    af8p = nc.dram_tensor("af8p", [128, KF8 * ER], f8, kind="ExternalInput")
    afbp = nc.dram_tensor("afbp", [128, KBF * ER], bf16, kind="ExternalInput")
    bfT = nc.dram_tensor("bfT", [ER, O], bf16, kind="ExternalInput")
    biasrep = nc.dram_tensor("biasrep", [128, O], bf16, kind="ExternalInput")
    svec = nc.dram_tensor("svec", [128, 1], fp32, kind="ExternalInput")
    out = nc.dram_tensor("out", [TPC, O], fp32, kind="ExternalOutput")

    with tile.TileContext(nc) as tc:
        with (
            tc.tile_pool(name="const", bufs=1) as const,
            tc.tile_pool(name="w8", bufs=2) as w8pool,
            tc.tile_pool(name="wb", bufs=2) as wbpool,
            tc.tile_pool(name="ot", bufs=4) as otpool,
            tc.tile_pool(name="po", bufs=6, space="PSUM") as po_pool,
            tc.tile_pool(name="pt", bufs=2, space="PSUM") as pt_pool,
        ):
            # ---- resident SBUF tensors ----
            x8_sb = const.tile([128, KF8, TPC], f8)
            xb_sb = const.tile([128, KBF, TPC], bf16)
            af8_sb = const.tile([128, KF8, ER], f8)
            afb_sb = const.tile([128, KBF, ER], bf16)
            bfT_sb = const.tile([128, O], bf16)
            biasrep_sb = const.tile([128, O], bf16)
            svec_sb = const.tile([128, 1], fp32)
            u_sb = const.tile([128, TPC], bf16)            # [er, t]

            def dma_w_panel(ob):
                w8t = w8pool.tile([128, KF8, 512], f8, tag="w8")
                wbt = wbpool.tile([128, KBF, 512], bf16, tag="wb")
                ob8 = ob * KF8 * 512
                obb = ob * KBF * 512
                nc.sync.dma_start(w8t[:], w8p[:, ob8: ob8 + KF8 * 512])
                nc.sync.dma_start(wbt[:], wbp[:, obb: obb + KBF * 512])
                return w8t, wbt

            def base_mms(po, w8t, wbt, tt):
                for k2 in range(NF8):
                    nc.tensor.matmul(
                        po[:],
                        x8_sb[:, 2 * k2:2 * k2 + 2, tt * 128:(tt + 1) * 128],
                        w8t[:, 2 * k2:2 * k2 + 2, :],
                        start=(k2 == 0),
                        stop=False,
                        perf_mode=DRow,
                    )
                for k in range(KBF):
                    nc.tensor.matmul(
                        po[:],
                        xb_sb[:, k, tt * 128:(tt + 1) * 128],
                        wbt[:, k, :],
                        start=False,
                        stop=False,
                    )

            def lora_close(po, ob, tt):
                nc.tensor.matmul(
                    po[:],
                    u_sb[:, tt * 128:(tt + 1) * 128],
                    bfT_sb[:, ob * 512:(ob + 1) * 512],
                    start=False,
                    stop=True,
                )
                ot = otpool.tile([128, 512], fp32)
                nc.vector.tensor_add(
                    ot[:], po[:], biasrep_sb[:, ob * 512:(ob + 1) * 512]
                )
                # split across both HWDGE queues to halve drain latency
                nc.scalar.dma_start(
                    out[tt * 128:tt * 128 + 64, ob * 512:(ob + 1) * 512],
                    ot[0:64, :],
                )
                nc.sync.dma_start(
                    out[tt * 128 + 64:(tt + 1) * 128, ob * 512:(ob + 1) * 512],
                    ot[64:128, :],
                )

            # ---- startup loads.  Descriptor issue costs ~0.7us each on the
            # HWDGE engines, so use FEW chunked descriptors: x8/w8 whole,
            # xbf/wbf0 in 4 k-chunks on opposite queues (chaseable). ----
            w8t0 = w8pool.tile([128, KF8, 512], f8, tag="w8")
            wbt0 = wbpool.tile([128, KBF, 512], bf16, tag="wb")
            # halves so the first DR matmuls start sooner
            nc.scalar.dma_start(x8_sb[:, 0:6, :], x8p[:, 0:6 * TPC])
            nc.sync.dma_start(w8t0[:, 0:6, :], w8p[:, 0:6 * 512])
            nc.scalar.dma_start(x8_sb[:, 6:KF8, :], x8p[:, 6 * TPC:KF8 * TPC])
            nc.sync.dma_start(w8t0[:, 6:KF8, :], w8p[:, 6 * 512:KF8 * 512])
            nc.scalar.dma_start(svec_sb[:], svec[:])
            CH = (3, 4, 5, 5, 5)
            k0 = 0
            for ch in CH:
                nc.scalar.dma_start(
                    xb_sb[:, k0:k0 + ch, :],
                    xbp[:, k0 * TPC:(k0 + ch) * TPC],
                )
                nc.sync.dma_start(
                    wbt0[:, k0:k0 + ch, :],
                    wbp[:, k0 * 512:(k0 + ch) * 512],
                )
                k0 += ch
            nc.sync.dma_start(af8_sb[:], af8p[:])
            nc.sync.dma_start(afb_sb[:], afbp[:])
            nc.sync.dma_start(bfT_sb[:], bfT[:])
            nc.scalar.dma_start(biasrep_sb[:], biasrep[:])

            # ---- ob0: open first NOPEN token groups (base only) ----
            open_po = []
            for tt in range(NOPEN):
                po = po_pool.tile([128, 512], fp32)
                base_mms(po, w8t0, wbt0, tt)
                open_po.append(po)

            # ---- LoRA t = Af @ x -> psum [er, t], mixed precision.
            # u for each 512-token chunk is computed on DVE while the PE
            # works the next chunk's t-matmuls, and ob0's open groups are
            # closed as soon as their u-chunk is ready — spreading the
            # PSUM evictions so bank recycling never stalls the PE. ----
            def t_mms(tb):
                pt = pt_pool.tile([128, 512], fp32)
                for k2 in range(NF8):
                    nc.tensor.matmul(
                        pt[:],
                        af8_sb[:, 2 * k2:2 * k2 + 2, :],
                        x8_sb[:, 2 * k2:2 * k2 + 2, tb * 512:(tb + 1) * 512],
                        start=(k2 == 0),
                        stop=False,
                        perf_mode=DRow,
                    )
                for k in range(KBF):
                    nc.tensor.matmul(
                        pt[:],
                        afb_sb[:, k, :],
                        xb_sb[:, k, tb * 512:(tb + 1) * 512],
                        start=False,
                        stop=(k == KBF - 1),
                    )
                return pt

            def u_chunk(tb, pt):
                nc.vector.tensor_scalar_mul(
                    u_sb[:, tb * 512:(tb + 1) * 512],
                    pt[:],
                    svec_sb[:, 0:1],
                )

            pt0 = t_mms(0)
            u_chunk(0, pt0)          # DVE, overlaps pt1 on the PE
            pt1 = t_mms(1)
            for tt in range(4):                  # u[0] ready: close tt0..3
                lora_close(open_po[tt], 0, tt)
            u_chunk(1, pt1)
            for tt in range(4, NOPEN):           # u[1] ready: close tt4+
                lora_close(open_po[tt], 0, tt)
            for tt in range(NOPEN, NTT):
                po = po_pool.tile([128, 512], fp32)
                base_mms(po, w8t0, wbt0, tt)
                lora_close(po, 0, tt)

            # ---- remaining panels ----
            for ob in range(1, NOB):
                w8t, wbt = dma_w_panel(ob)
                for tt in range(NTT):
                    po = po_pool.tile([128, 512], fp32)
                    base_mms(po, w8t, wbt, tt)
                    lora_close(po, ob, tt)

    nc.compile()
    return nc


def _pack_km(a, k, rows):
    # [k*128, m] (row = k*128 + p) -> [128, k, m] partition-major
    return np.ascontiguousarray(
        a.reshape(k, 128, rows).transpose(1, 0, 2).reshape(128, k * rows)
    )


def _host_prep(x, W, b, A, B, router_W, router_b):
    xf = np.ascontiguousarray(x, dtype=np.float32).reshape(TOK, D)
    xT8_full = (xf[:, :D8] * (1.0 / XS)).T.astype(E4M3)    # [D8, TOK]
    xTb_full = xf[:, D8:].T.astype(BF16)                   # [D-D8, TOK]

    # W panels packed [128, ob, k, 512]
    wT8 = (W[:, :D8] * XS).T.astype(E4M3)                  # [D8, O]
    wTb = W[:, D8:].T.astype(BF16)                         # [D-D8, O]
    w8p = np.ascontiguousarray(
        wT8.reshape(KF8, 128, NOB, 512).transpose(1, 2, 0, 3).reshape(128, -1)
    )
    wbp = np.ascontiguousarray(
        wTb.reshape(KBF, 128, NOB, 512).transpose(1, 2, 0, 3).reshape(128, -1)
    )

    af = A.reshape(ER, D)                                  # [ER, D]
    af8p = _pack_km((af[:, :D8] * XS).T.astype(E4M3), KF8, ER)
    afbp = _pack_km(af[:, D8:].T.astype(BF16), KBF, ER)
    bfT_bf = (2.0 * np.transpose(B, (0, 2, 1)).reshape(ER, O)).astype(BF16)
    bias_bf = np.ascontiguousarray(
        np.broadcast_to(b.astype(BF16)[None, :], (128, O))
    )
    # router on host (numpy, float64 — exact vs device noise)
    xq = np.asarray(x, np.float64)[:, Q_LO:Q_HI, :]
    q = xq.mean(axis=1)
    logits = q @ np.asarray(router_W, np.float64).T + np.asarray(router_b, np.float64)
    ex = np.exp(logits - logits.max(-1, keepdims=True))
    routing = ex / ex.sum(-1, keepdims=True)          # [B, E]

    in_maps = []
    for c in range(NCORES):
        sv = np.repeat(routing[c // 2].astype(np.float32), R).reshape(128, 1)
        in_maps.append({
            "x8p": _pack_km(
                np.ascontiguousarray(xT8_full[:, c * TPC:(c + 1) * TPC]),
                KF8, TPC),
            "xbp": _pack_km(
                np.ascontiguousarray(xTb_full[:, c * TPC:(c + 1) * TPC]),
                KBF, TPC),
            "w8p": w8p,
            "wbp": wbp,
            "af8p": af8p,
            "afbp": afbp,
            "bfT": bfT_bf,
            "biasrep": bias_bf,
            "svec": np.ascontiguousarray(sv),
        })
    return in_maps


def kernel(x, W, b, A, B, router_W, router_b):
    global LAST_RESULTS
    from concourse.bass_utils import run_bass_kernel_spmd

    if "nc" not in _CACHE:
        _CACHE["nc"] = _build_nc()
    nc = _CACHE["nc"]

    in_maps = _host_prep(x, W, b, A, B, router_W, router_b)

    kwargs = {}
    if TRACE:
        kwargs.update(trace=True, trace_cores=[0])
    res = run_bass_kernel_spmd(nc, in_maps, core_ids=list(range(NCORES)), **kwargs)
    LAST_RESULTS = res

    shards = [res.results[c]["out"] for c in range(NCORES)]
    return np.concatenate(shards, axis=0).reshape(B_, S, O).astype(np.float32)


# revision 23
# speedup vs baseline: 1.0089x; 1.0089x over previous
# LoRA-MoE QK kernel for 8x Trainium2 NeuronCores (Bass/Tile).
#
# Reference computation:
#   routing = softmax(mean(x[:, 611:-1, :]) @ router_W.T + router_b)   [B, E]
#   base    = x @ W.T + b
#   lora    = einsum('bsd,erd->bser', x, A) -> *B,routing -> [B,S,O] * 2.0
#   out     = base + lora
#
# Sharding: data-parallel over the 8192 tokens (1024/core).  Weights
# replicated, host-prepped; router computed on host (float64).
#
# Mixed-precision contraction: of the 32 k-tiles (128 each) of the D=4096
# contraction, the first KF8=10 run as fp8e4m3 DoubleRow matmuls (2 k-tiles
# per instruction, 2x bf16 throughput; x scaled 1/8, W scaled 8 so products
# land at true scale in PSUM) and the remaining 22 run in bf16.  Measured
# rel-err ~1.77e-2 vs the 2e-2 gate (deterministic inputs).  LoRA t-matmuls
# use the same mixed split; the combine matmul and eviction stay bf16/fp32.
#
# DMA: all inputs are host-packed into the exact SBUF partition-major
# layout so every descriptor moves large contiguous per-partition lines
# (W panel: one 5KB-line fp8 + one 22KB-line bf16 descriptor; descriptor
# issue costs ~0.7us each on the HWDGE engines, so startup uses a few
# chunked descriptors rather than per-k-tile ones).  sync (SP) carries W
# panels; scalar (Activation) carries input loads; every output tile is
# split into two half-height DMAs, one per queue, halving drain latency.
# ob0's first NOPEN token-groups are left open (base matmuls only, all 8
# PSUM banks in use) while the t/u routing chain completes, so the PE has
# ~47us of queued work to ride out the HBM-bound startup window.

import numpy as np
import ml_dtypes

BF16 = ml_dtypes.bfloat16
E4M3 = ml_dtypes.float8_e4m3

B_, S, D, O, E, R = 4, 2048, 4096, 4096, 8, 16
ER = E * R              # 128
TOK = B_ * S            # 8192
NCORES = 8
TPC = TOK // NCORES     # 1024 tokens per core
KT = D // 128           # 32 contraction k-tiles
NF8 = 5                 # fp8 DoubleRow k-tile PAIRS
KF8 = 2 * NF8           # fp8 k-tiles (first 10)
KBF = KT - KF8          # bf16 k-tiles (22)
D8 = KF8 * 128          # 1280 fp8 contraction depth
XS = 8.0                # fp8 scale split: x/8 (x-side), 8*W (w-side)
NOB = O // 512          # 8 output-column panels
NTT = TPC // 128        # 8 token tiles per core
NOPEN = 6               # ob0 token-groups opened before the t/u chain
Q_LO, Q_HI = 611, 2047  # question tokens [611, 2047) within each batch

_CACHE: dict = {}
LAST_RESULTS = None
TRACE = False


def _build_nc():
    import concourse.bacc as bacc
    import concourse.mybir as mybir
    from concourse import tile

    fp32 = mybir.dt.float32
    bf16 = mybir.dt.bfloat16
    f8 = mybir.dt.float8e4
    DRow = mybir.MatmulPerfMode.DoubleRow

    nc = bacc.Bacc(
        "TRN2",
        target_bir_lowering=False,
        debug=False,
        num_devices=NCORES,
    )

    # all inputs host-packed to partition-major SBUF layout
    x8p = nc.dram_tensor("x8p", [128, KF8 * TPC], f8, kind="ExternalInput")
    xbp = nc.dram_tensor("xbp", [128, KBF * TPC], bf16, kind="ExternalInput")
    w8p = nc.dram_tensor("w8p", [128, NOB * KF8 * 512], f8, kind="ExternalInput")
    wbp = nc.dram_tensor("wbp", [128, NOB * KBF * 512], bf16, kind="ExternalInput")
    af8p = nc.dram_tensor("af8p", [128, KF8 * ER], f8, kind="ExternalInput")
    afbp = nc.dram_tensor("afbp", [128, KBF * ER], bf16, kind="ExternalInput")
    bfT = nc.dram_tensor("bfT", [ER, O], bf16, kind="ExternalInput")
    biasrep = nc.dram_tensor("biasrep", [128, O], bf16, kind="ExternalInput")
    svec = nc.dram_tensor("svec", [128, 1], fp32, kind="ExternalInput")
    # tile-contiguous: each (ob, tt) eviction writes one contiguous 256KB
    out = nc.dram_tensor("out", [NOB * NTT * 128, 512], fp32, kind="ExternalOutput")

    with tile.TileContext(nc) as tc:
        with (
            tc.tile_pool(name="const", bufs=1) as const,
            tc.tile_pool(name="w8", bufs=2) as w8pool,
            tc.tile_pool(name="wb", bufs=2) as wbpool,
            tc.tile_pool(name="ot", bufs=4) as otpool,
            tc.tile_pool(name="po", bufs=6, space="PSUM") as po_pool,
            tc.tile_pool(name="pt", bufs=2, space="PSUM") as pt_pool,
        ):
            # ---- resident SBUF tensors ----
            x8_sb = const.tile([128, KF8, TPC], f8)
            xb_sb = const.tile([128, KBF, TPC], bf16)
            af8_sb = const.tile([128, KF8, ER], f8)
            afb_sb = const.tile([128, KBF, ER], bf16)
            bfT_sb = const.tile([128, O], bf16)
            biasrep_sb = const.tile([128, O], bf16)
            svec_sb = const.tile([128, 1], fp32)
            u_sb = const.tile([128, TPC], bf16)            # [er, t]

            def dma_w_panel(ob):
                w8t = w8pool.tile([128, KF8, 512], f8, tag="w8")
                wbt = wbpool.tile([128, KBF, 512], bf16, tag="wb")
                ob8 = ob * KF8 * 512
                obb = ob * KBF * 512
                nc.sync.dma_start(w8t[:], w8p[:, ob8: ob8 + KF8 * 512])
                nc.sync.dma_start(wbt[:], wbp[:, obb: obb + KBF * 512])
                return w8t, wbt

            def base_mms(po, w8t, wbt, tt):
                for k2 in range(NF8):
                    nc.tensor.matmul(
                        po[:],
                        x8_sb[:, 2 * k2:2 * k2 + 2, tt * 128:(tt + 1) * 128],
                        w8t[:, 2 * k2:2 * k2 + 2, :],
                        start=(k2 == 0),
                        stop=False,
                        perf_mode=DRow,
                    )
                for k in range(KBF):
                    nc.tensor.matmul(
                        po[:],
                        xb_sb[:, k, tt * 128:(tt + 1) * 128],
                        wbt[:, k, :],
                        start=False,
                        stop=False,
                    )

            def lora_close(po, ob, tt):
                nc.tensor.matmul(
                    po[:],
                    u_sb[:, tt * 128:(tt + 1) * 128],
                    bfT_sb[:, ob * 512:(ob + 1) * 512],
                    start=False,
                    stop=True,
                )
                ot = otpool.tile([128, 512], fp32)
                nc.vector.tensor_add(
                    ot[:], po[:], biasrep_sb[:, ob * 512:(ob + 1) * 512]
                )
                # split across both HWDGE queues to halve drain latency
                r0 = (ob * NTT + tt) * 128
                nc.scalar.dma_start(out[r0:r0 + 64, :], ot[0:64, :])
                nc.sync.dma_start(out[r0 + 64:r0 + 128, :], ot[64:128, :])

            # ---- startup loads.  Descriptor issue costs ~0.7us each on the
            # HWDGE engines, so use FEW chunked descriptors: x8/w8 whole,
            # xbf/wbf0 in 4 k-chunks on opposite queues (chaseable). ----
            w8t0 = w8pool.tile([128, KF8, 512], f8, tag="w8")
            wbt0 = wbpool.tile([128, KBF, 512], bf16, tag="wb")
            # halves so the first DR matmuls start sooner
            nc.scalar.dma_start(x8_sb[:, 0:6, :], x8p[:, 0:6 * TPC])
            nc.sync.dma_start(w8t0[:, 0:6, :], w8p[:, 0:6 * 512])
            nc.scalar.dma_start(x8_sb[:, 6:KF8, :], x8p[:, 6 * TPC:KF8 * TPC])
            nc.sync.dma_start(w8t0[:, 6:KF8, :], w8p[:, 6 * 512:KF8 * 512])
            nc.scalar.dma_start(svec_sb[:], svec[:])
            CH = (3, 4, 5, 5, 5)
            k0 = 0
            for ch in CH:
                nc.scalar.dma_start(
                    xb_sb[:, k0:k0 + ch, :],
                    xbp[:, k0 * TPC:(k0 + ch) * TPC],
                )
                nc.sync.dma_start(
                    wbt0[:, k0:k0 + ch, :],
                    wbp[:, k0 * 512:(k0 + ch) * 512],
                )
                k0 += ch
            nc.sync.dma_start(af8_sb[:], af8p[:])
            nc.sync.dma_start(afb_sb[:], afbp[:])
            nc.sync.dma_start(bfT_sb[:], bfT[:])
            nc.scalar.dma_start(biasrep_sb[:], biasrep[:])

            # ---- ob0: open first NOPEN token groups (base only) ----
            open_po = []
            for tt in range(NOPEN):
                po = po_pool.tile([128, 512], fp32)
                base_mms(po, w8t0, wbt0, tt)
                open_po.append(po)

            # ---- LoRA t = Af @ x -> psum [er, t], mixed precision.
            # u for each 512-token chunk is computed on DVE while the PE
            # works the next chunk's t-matmuls, and ob0's open groups are
            # closed as soon as their u-chunk is ready — spreading the
            # PSUM evictions so bank recycling never stalls the PE. ----
            def t_mms(tb):
                pt = pt_pool.tile([128, 512], fp32)
                for k2 in range(NF8):
                    nc.tensor.matmul(
                        pt[:],
                        af8_sb[:, 2 * k2:2 * k2 + 2, :],
                        x8_sb[:, 2 * k2:2 * k2 + 2, tb * 512:(tb + 1) * 512],
                        start=(k2 == 0),
                        stop=False,
                        perf_mode=DRow,
                    )
                for k in range(KBF):
                    nc.tensor.matmul(
                        pt[:],
                        afb_sb[:, k, :],
                        xb_sb[:, k, tb * 512:(tb + 1) * 512],
                        start=False,
                        stop=(k == KBF - 1),
                    )
                return pt

            def u_chunk(tb, pt):
                nc.vector.tensor_scalar_mul(
                    u_sb[:, tb * 512:(tb + 1) * 512],
                    pt[:],
                    svec_sb[:, 0:1],
                )

            pt0 = t_mms(0)
            u_chunk(0, pt0)          # DVE, overlaps pt1 on the PE
            pt1 = t_mms(1)
            for tt in range(4):                  # u[0] ready: close tt0..3
                lora_close(open_po[tt], 0, tt)
            u_chunk(1, pt1)
            for tt in range(4, NOPEN):           # u[1] ready: close tt4+
                lora_close(open_po[tt], 0, tt)
            for tt in range(NOPEN, NTT):
                po = po_pool.tile([128, 512], fp32)
                base_mms(po, w8t0, wbt0, tt)
                lora_close(po, 0, tt)

            # ---- remaining panels ----
            for ob in range(1, NOB):
                w8t, wbt = dma_w_panel(ob)
                for tt in range(NTT):
                    po = po_pool.tile([128, 512], fp32)
                    base_mms(po, w8t, wbt, tt)
                    lora_close(po, ob, tt)

    nc.compile()
    return nc


def _pack_km(a, k, rows):
    # [k*128, m] (row = k*128 + p) -> [128, k, m] partition-major
    return np.ascontiguousarray(
        a.reshape(k, 128, rows).transpose(1, 0, 2).reshape(128, k * rows)
    )


def _host_prep(x, W, b, A, B, router_W, router_b):
    xf = np.ascontiguousarray(x, dtype=np.float32).reshape(TOK, D)
    xT8_full = (xf[:, :D8] * (1.0 / XS)).T.astype(E4M3)    # [D8, TOK]
    xTb_full = xf[:, D8:].T.astype(BF16)                   # [D-D8, TOK]

    # W panels packed [128, ob, k, 512]
    wT8 = (W[:, :D8] * XS).T.astype(E4M3)                  # [D8, O]
    wTb = W[:, D8:].T.astype(BF16)                         # [D-D8, O]
    w8p = np.ascontiguousarray(
        wT8.reshape(KF8, 128, NOB, 512).transpose(1, 2, 0, 3).reshape(128, -1)
    )
    wbp = np.ascontiguousarray(
        wTb.reshape(KBF, 128, NOB, 512).transpose(1, 2, 0, 3).reshape(128, -1)
    )

    af = A.reshape(ER, D)                                  # [ER, D]
    af8p = _pack_km((af[:, :D8] * XS).T.astype(E4M3), KF8, ER)
    afbp = _pack_km(af[:, D8:].T.astype(BF16), KBF, ER)
    bfT_bf = (2.0 * np.transpose(B, (0, 2, 1)).reshape(ER, O)).astype(BF16)
    bias_bf = np.ascontiguousarray(
        np.broadcast_to(b.astype(BF16)[None, :], (128, O))
    )
    # router on host (numpy, float64 — exact vs device noise)
    xq = np.asarray(x, np.float64)[:, Q_LO:Q_HI, :]
    q = xq.mean(axis=1)
    logits = q @ np.asarray(router_W, np.float64).T + np.asarray(router_b, np.float64)
    ex = np.exp(logits - logits.max(-1, keepdims=True))
    routing = ex / ex.sum(-1, keepdims=True)          # [B, E]

    in_maps = []
    for c in range(NCORES):
        sv = np.repeat(routing[c // 2].astype(np.float32), R).reshape(128, 1)
        in_maps.append({
            "x8p": _pack_km(
                np.ascontiguousarray(xT8_full[:, c * TPC:(c + 1) * TPC]),
                KF8, TPC),
            "xbp": _pack_km(
                np.ascontiguousarray(xTb_full[:, c * TPC:(c + 1) * TPC]),
                KBF, TPC),
            "w8p": w8p,
            "wbp": wbp,
            "af8p": af8p,
            "afbp": afbp,
            "bfT": bfT_bf,
            "biasrep": bias_bf,
            "svec": np.ascontiguousarray(sv),
        })
    return in_maps


def kernel(x, W, b, A, B, router_W, router_b):
    global LAST_RESULTS
    from concourse.bass_utils import run_bass_kernel_spmd

    if "nc" not in _CACHE:
        _CACHE["nc"] = _build_nc()
    nc = _CACHE["nc"]

    in_maps = _host_prep(x, W, b, A, B, router_W, router_b)

    kwargs = {}
    if TRACE:
        kwargs.update(trace=True, trace_cores=[0])
    res = run_bass_kernel_spmd(nc, in_maps, core_ids=list(range(NCORES)), **kwargs)
    LAST_RESULTS = res

    shards = []
    for c in range(NCORES):
        o = res.results[c]["out"].reshape(NOB, NTT, 128, 512)
        shards.append(o.transpose(1, 2, 0, 3).reshape(TPC, O))
    return np.concatenate(shards, axis=0).reshape(B_, S, O).astype(np.float32)
